# revision 2
# baseline (speedup 1.0000x reference)
"""TT-adapter linear kernel for TRN2, data-parallel over batch on 8 NeuronCores.

Math: out = x @ W.T + b + ALPHA * TT(x) where TT is a tensor-train
factorized linear map (6 small cores).  TT is linear in x, so the whole
module collapses to a single matmul with a merged weight:

    T[o, d]  = TT-matrix reconstruction (1024x1024, ~17 MFLOP to build)
    Wc       = W + ALPHA * T
    out      = x @ Wc.T + b

Host folds the tiny TT cores into Wc (0.05% of total FLOPs); the 34 GFLOP
batched matmul runs on device, one batch element per NeuronCore.

Device layout (per core, P=128 partitions):
    xt  bf16 [128, 8, 2048]  xt[p, d, s] = x[b, s, 128*d + p]   (contraction on partitions)
    wt  bf16 [128, 8, 1024]  wt[p, d, o] = Wc[o, 128*d + p]
    bi  f32  [128, 8]        bi[p, oo]   = b[128*oo + p]
    out f32  [128, 8, 2048]  out[p, oo, s] = result[b, s, 128*oo + p]

Kernel: for each o-tile (8) and s-chunk (4): accumulate 8 matmuls over the
d-tiles into one PSUM bank ([o 128] x [s 512]), evict via ScalarE with the
per-partition bias add, DMA back.
"""

import numpy as np
import ml_dtypes

import concourse.bass as bass  # noqa: F401  (registers engines)
import concourse.mybir as mybir
import concourse.tile as tile
from concourse import bacc
from concourse.bass_utils import run_bass_kernel_spmd

ALPHA = 16.0
B, S, D = 8, 2048, 1024
P = 128
DO = D // P          # 8 contraction tiles
OO = D // P          # 8 output-row tiles
SCH = 512            # s-chunk (PSUM bank free dim)
NS = S // SCH        # 4 s-chunks

_NC = None


def _build_nc():
    nc = bacc.Bacc("TRN2", target_bir_lowering=False, debug=False)
    xt = nc.declare_dram_parameter("xt", [P, DO, S], mybir.dt.bfloat16, isOutput=False)
    wt = nc.declare_dram_parameter("wt", [P, DO, D], mybir.dt.bfloat16, isOutput=False)
    bi = nc.declare_dram_parameter("bi", [P, OO], mybir.dt.float32, isOutput=False)
    out = nc.declare_dram_parameter("out", [P, OO, S], mybir.dt.float32, isOutput=True)

    with tile.TileContext(nc) as tc:
        with tc.tile_pool(name="wp", bufs=DO) as wp, \
             tc.tile_pool(name="xp", bufs=DO * NS) as xp, \
             tc.tile_pool(name="cp", bufs=1) as cp, \
             tc.tile_pool(name="op", bufs=6) as op_, \
             tc.tile_pool(name="pp", bufs=8, space="PSUM") as pp:
            bias_sb = cp.tile([P, OO], mybir.dt.float32, name="bias_sb")
            nc.sync.dma_start(bias_sb[:], bi[:, :])
            # inputs on the SP HWDGE queue, split so the first accumulation
            # chains can start as early as possible
            w_sb = []
            x_sb = [[None] * NS for _ in range(DO)]
            for d in range(DO):
                wtile = wp.tile([P, D], mybir.dt.bfloat16, name=f"w{d}", tag="w")
                nc.sync.dma_start(wtile[:], wt[:, d, :])
                w_sb.append(wtile)
                xtile = xp.tile([P, SCH], mybir.dt.bfloat16, name=f"x{d}_0", tag="x")
                nc.sync.dma_start(xtile[:], xt[:, d, 0:SCH])
                x_sb[d][0] = xtile
            for sc in range(1, NS):
                for d in range(DO):
                    xtile = xp.tile([P, SCH], mybir.dt.bfloat16, name=f"x{d}_{sc}", tag="x")
                    nc.sync.dma_start(xtile[:], xt[:, d, sc * SCH:(sc + 1) * SCH])
                    x_sb[d][sc] = xtile
            for o in range(OO):
                for sc in range(NS):
                    ps = pp.tile([P, SCH], mybir.dt.float32, name=f"ps{o}_{sc}", tag="ps")
                    for d in range(DO):
                        nc.tensor.matmul(
                            ps[:],
                            w_sb[d][:, o * P:(o + 1) * P],
                            x_sb[d][sc][:],
                            start=(d == 0),
                            stop=(d == DO - 1),
                        )
                    ot = op_.tile([P, SCH], mybir.dt.float32, name=f"ot{o}_{sc}", tag="ot")
                    # evict on DVE (PSUM read + per-partition bias bcast add)
                    nc.vector.tensor_tensor(
                        ot[:], ps[:],
                        bias_sb[:, o:o + 1].to_broadcast((P, SCH)),
                        mybir.AluOpType.add,
                    )
                    # outputs on the Activation HWDGE queue (separate from inputs)
                    nc.scalar.dma_start(out[:, o, sc * SCH:(sc + 1) * SCH], ot[:])

    nc.compile()
    return nc


def _get_nc():
    global _NC
    if _NC is None:
        _NC = _build_nc()
    return _NC


def _merged_weight_T(W, b, core0, core1, core2, core3, core4, core5):
    """Wc.T[d, o] = W.T + ALPHA * (TT matrix).T, float32."""
    f8 = np.float64
    # m-side Phi[d, p3] with d = m3*128 + m2*8 + m1
    A = core0[0].astype(f8)                                   # (m1, p1)
    Bm = np.einsum('ap,pbq->abq', A, core1.astype(f8))        # (m1, m2, p2)
    C = np.einsum('abq,qcr->abcr', Bm, core2.astype(f8))      # (m1, m2, m3, p3)
    Phi = C.transpose(2, 1, 0, 3).reshape(D, 8)               # (d, p3)
    # n-side Psi[p3, o] with o = n1*128 + n2*8 + n3
    Dn = np.einsum('paq,qbr->pabr', core3.astype(f8), core4.astype(f8))
    E = np.einsum('pabq,qc->pabc', Dn, core5[:, :, 0].astype(f8))
    Psi = E.reshape(8, D)                                     # (p3, o)
    WcT = W.T.astype(f8) + ALPHA * (Phi @ Psi)                # (d, o)
    return WcT.astype(np.float32)


def _prep_in_maps(x, W, b, core0, core1, core2, core3, core4, core5):
    WcT = _merged_weight_T(W, b, core0, core1, core2, core3, core4, core5)
    wt = WcT.reshape(DO, P, D).transpose(1, 0, 2).astype(ml_dtypes.bfloat16)
    bi = np.ascontiguousarray(b.reshape(OO, P).T).astype(np.float32)
    in_maps = []
    for bb in range(B):
        xt = x[bb].T.reshape(DO, P, S).transpose(1, 0, 2).astype(ml_dtypes.bfloat16)
        in_maps.append({"xt": xt, "wt": wt, "bi": bi})
    return in_maps


def _gather(results):
    outs = []
    for bb in range(B):
        o = np.asarray(results[bb]["out"])          # [P, OO, S] f32
        outs.append(o.transpose(2, 1, 0).reshape(S, D))
    return np.ascontiguousarray(np.stack(outs)).astype(np.float32)


def run(inputs, **spmd_kwargs):
    """Run on 8 cores; returns (full_output, BassKernelResults)."""
    in_maps = _prep_in_maps(**inputs)
    nc = _get_nc()
    res = run_bass_kernel_spmd(nc, in_maps, core_ids=list(range(B)), **spmd_kwargs)
    return _gather(res.results), res


def kernel(x, W, b, core0, core1, core2, core3, core4, core5):
    out, _ = run(dict(x=x, W=W, b=b, core0=core0, core1=core1, core2=core2,
                      core3=core3, core4=core4, core5=core5))
    return out


# revision 3
# speedup vs baseline: 1.2838x; 1.2838x over previous
"""TT-adapter linear kernel for TRN2, data-parallel over batch on 8 NeuronCores.

Math: out = x @ W.T + b + ALPHA * TT(x) where TT is a tensor-train
factorized linear map (6 small cores).  TT is linear in x, so the whole
module collapses to a single matmul with a merged weight:

    T[o, d]  = TT-matrix reconstruction (1024x1024, ~17 MFLOP to build)
    Wc       = W + ALPHA * T
    out      = x @ Wc.T + b

Host folds the tiny TT cores into Wc (0.05% of total FLOPs); the 34 GFLOP
batched matmul runs on device, one batch element per NeuronCore.

Device layout (per core, P=128 partitions), all blocks DRAM-contiguous:
    xt  bf16 [4, 128, 2, 2048]  xt[c, p, j, s] = x[b, s, 128*(2c+j) + p]
    wt  bf16 [2, 128, 4, 1024]  wt[c, p, j, o] = Wc[o, 128*(4c+j) + p]
    bi  f32  [128, 8]           bi[p, oo]      = b[128*oo + p]
    out f32  [8, 128, 2048]     out[oo, p, s]  = result[b, s, 128*oo + p]

Kernel: for each o-tile (8) and s-chunk (4): accumulate 8 matmuls over the
d-tiles into one PSUM bank ([o 128] x [s 512]), evict via ScalarE with the
per-partition bias add, DMA back.
"""

import numpy as np
import ml_dtypes

import concourse.bass as bass  # noqa: F401  (registers engines)
import concourse.mybir as mybir
import concourse.tile as tile
from concourse import bacc
from concourse.bass_utils import run_bass_kernel_spmd

ALPHA = 16.0
B, S, D = 8, 2048, 1024
P = 128
DO = D // P          # 8 contraction tiles
OO = D // P          # 8 output-row tiles
SCH = 512            # s-chunk (PSUM bank free dim)
NS = S // SCH        # 4 s-chunks
XC = 4               # x DMA chunks (2 d-tiles each, 1MB)
WC = 2               # w DMA chunks (4 d-tiles each, 1MB)

_NC = None


def _build_nc():
    nc = bacc.Bacc("TRN2", target_bir_lowering=False, debug=False)
    xt = nc.declare_dram_parameter("xt", [XC, P, DO // XC, S], mybir.dt.bfloat16, isOutput=False)
    wt = nc.declare_dram_parameter("wt", [WC, P, DO // WC, D], mybir.dt.bfloat16, isOutput=False)
    bi = nc.declare_dram_parameter("bi", [P, OO], mybir.dt.float32, isOutput=False)
    out = nc.declare_dram_parameter("out", [OO, P, S], mybir.dt.float32, isOutput=True)

    with tile.TileContext(nc) as tc:
        with tc.tile_pool(name="wp", bufs=WC) as wp, \
             tc.tile_pool(name="xp", bufs=XC) as xp, \
             tc.tile_pool(name="cp", bufs=1) as cp, \
             tc.tile_pool(name="op", bufs=6) as op_, \
             tc.tile_pool(name="pp", bufs=8, space="PSUM") as pp:
            bias_sb = cp.tile([P, OO], mybir.dt.float32, name="bias_sb")
            nc.sync.dma_start(bias_sb[:], bi[:, :])
            w_ch = []
            x_ch = []
            for c in range(WC):
                wtile = wp.tile([P, DO // WC, D], mybir.dt.bfloat16, name=f"w{c}", tag="w")
                nc.sync.dma_start(wtile[:], wt[c])
                w_ch.append(wtile)
            for c in range(XC):
                xtile = xp.tile([P, DO // XC, S], mybir.dt.bfloat16, name=f"x{c}", tag="x")
                nc.sync.dma_start(xtile[:], xt[c])
                x_ch.append(xtile)

            def w_ap(d, o):  # [128, 128] lhsT slice for d-tile, o-tile
                return w_ch[d // (DO // WC)][:, d % (DO // WC), o * P:(o + 1) * P]

            def x_ap(d, sc):  # [128, 512] rhs slice
                return x_ch[d // (DO // XC)][:, d % (DO // XC), sc * SCH:(sc + 1) * SCH]

            for o in range(OO):
                for sc in range(NS):
                    ps = pp.tile([P, SCH], mybir.dt.float32, name=f"ps{o}_{sc}", tag="ps")
                    for d in range(DO):
                        nc.tensor.matmul(
                            ps[:], w_ap(d, o), x_ap(d, sc),
                            start=(d == 0), stop=(d == DO - 1),
                        )
                    ot = op_.tile([P, SCH], mybir.dt.float32, name=f"ot{o}_{sc}", tag="ot")
                    nc.scalar.add(ot[:], ps[:], bias_sb[:, o:o + 1])
                    nc.sync.dma_start(out[o, :, sc * SCH:(sc + 1) * SCH], ot[:])

    nc.compile()
    return nc


def _get_nc():
    global _NC
    if _NC is None:
        _NC = _build_nc()
    return _NC


def _merged_weight_T(W, b, core0, core1, core2, core3, core4, core5):
    """Wc.T[d, o] = W.T + ALPHA * (TT matrix).T, float32."""
    f8 = np.float64
    # m-side Phi[d, p3] with d = m3*128 + m2*8 + m1
    A = core0[0].astype(f8)                                   # (m1, p1)
    Bm = np.einsum('ap,pbq->abq', A, core1.astype(f8))        # (m1, m2, p2)
    C = np.einsum('abq,qcr->abcr', Bm, core2.astype(f8))      # (m1, m2, m3, p3)
    Phi = C.transpose(2, 1, 0, 3).reshape(D, 8)               # (d, p3)
    # n-side Psi[p3, o] with o = n1*128 + n2*8 + n3
    Dn = np.einsum('paq,qbr->pabr', core3.astype(f8), core4.astype(f8))
    E = np.einsum('pabq,qc->pabc', Dn, core5[:, :, 0].astype(f8))
    Psi = E.reshape(8, D)                                     # (p3, o)
    WcT = W.T.astype(f8) + ALPHA * (Phi @ Psi)                # (d, o)
    return WcT.astype(np.float32)


def _prep_in_maps(x, W, b, core0, core1, core2, core3, core4, core5):
    WcT = _merged_weight_T(W, b, core0, core1, core2, core3, core4, core5)
    # wt[c, p, j, o] = WcT[128*(4c+j) + p, o]
    wt = WcT.reshape(WC, DO // WC, P, D).transpose(0, 2, 1, 3).astype(ml_dtypes.bfloat16)
    bi = np.ascontiguousarray(b.reshape(OO, P).T).astype(np.float32)
    in_maps = []
    for bb in range(B):
        # xt[c, p, j, s] = x[b, s, 128*(2c+j) + p]
        xTd = x[bb].T  # (d, s)
        xt = xTd.reshape(XC, DO // XC, P, S).transpose(0, 2, 1, 3).astype(ml_dtypes.bfloat16)
        in_maps.append({"xt": xt, "wt": wt, "bi": bi})
    return in_maps


def _gather(results):
    outs = []
    for bb in range(B):
        o = np.asarray(results[bb]["out"])          # [OO, P, S] f32
        outs.append(o.transpose(2, 0, 1).reshape(S, D))
    return np.ascontiguousarray(np.stack(outs)).astype(np.float32)


def run(inputs, **spmd_kwargs):
    """Run on 8 cores; returns (full_output, BassKernelResults)."""
    in_maps = _prep_in_maps(**inputs)
    nc = _get_nc()
    res = run_bass_kernel_spmd(nc, in_maps, core_ids=list(range(B)), **spmd_kwargs)
    return _gather(res.results), res


def kernel(x, W, b, core0, core1, core2, core3, core4, core5):
    out, _ = run(dict(x=x, W=W, b=b, core0=core0, core1=core1, core2=core2,
                      core3=core3, core4=core4, core5=core5))
    return out
